# revision 12
# baseline (speedup 1.0000x reference)
"""Trainium2 Bass kernel for nn_CrossAttentionGraphBlock.

Strategy (hardcoded for B=16, NQ=512, NK=1024, D=768, L=512, H=12, DH=64):
 - Data-parallel over batch: 8 cores x 2 batches each. No collectives.
 - Host-side (numpy, cheap): fuse the outer q/k/v projections with the
   in-proj of MultiheadAttention (both are linear):
       qWe = qW @ in_qW / sqrt(DH)   (softmax scale folded in)
       kWe = kW @ in_kW,  vWe = vW @ in_vW  (+ fused biases)
   and pre-transpose activations so the device never transposes inputs.
 - On-chip dataflow is feature-major ([feature, token] in SBUF) end to end:
   projections, attention, out-proj, both layernorms.  Per head h:
       S^T[k,q]  = (kh_h)^T.T @ qh_h^T          (K=DH=64 contraction)
       P^T       = exp(S^T)                      (no max-sub needed: |S|<~1)
       ctx~aug^T = vh_aug.T @ P^T                (65th row = softmax denom)
   then ctx^T rows are scaled by 1/denom (PE broadcast of the reciprocal).
   Key-padding mask is applied by zeroing masked rows of vh_aug (incl. the
   ones-column), which removes masked keys from both ctx and the denom.
 - Heads are processed in pairs with interleaved S matmuls at partition
   bases 0/64 -> tile_position row groups (0,0)/(64,0) run concurrently.
 - LayerNorm stats across features (= partitions) via ones-column matmuls.
 - Final [feat,tok] -> [tok,feat] transpose on the tensor engine.
 - Precision: matmuls bf16 (fp32 PSUM accumulate); LN stats, softmax
   denominators and the final LN2 output stay fp32.
"""

import numpy as np
import ml_dtypes

import concourse.bass as bass
import concourse.mybir as mybir
import concourse.tile as tile
from concourse import bacc
from concourse.bass_utils import run_bass_kernel_spmd

P = 128
B, NQ, NK, D, L, H = 16, 512, 1024, 768, 512, 12
DH = D // H          # 64
NCORES = 8
BL = B // NCORES     # 2 batches per core
EPS = 1e-5
F32 = mybir.dt.float32
BF16 = mybir.dt.bfloat16
NPBF = ml_dtypes.bfloat16
AF = mybir.ActivationFunctionType
ALU = mybir.AluOpType

KD = D // P          # 6 chunks of the D (=768) contraction/feature dim
KL = L // P          # 4 chunks of the L (=512) contraction dim
MQ = NQ // P         # 4 query-token tiles
MK = NK // P         # 8 key-token tiles
VA = H * (DH + 1)    # 780: vh augmented with one ones-column per head

_NC_CACHE = {}


def _build_nc():
    nc = bacc.Bacc()

    gnT = nc.declare_dram_parameter("gnT", [BL, D, NQ], BF16, isOutput=False)
    gnTf = nc.declare_dram_parameter("gnTf", [BL, D, NQ], F32, isOutput=False)
    condT = nc.declare_dram_parameter("condT", [BL, L, NK], BF16, isOutput=False)
    qWe = nc.declare_dram_parameter("qWe", [D, D], BF16, isOutput=False)
    kWe = nc.declare_dram_parameter("kWe", [L, D], BF16, isOutput=False)
    vWe = nc.declare_dram_parameter("vWe", [L, VA], BF16, isOutput=False)
    outW = nc.declare_dram_parameter("outW", [D, D], BF16, isOutput=False)
    d1W = nc.declare_dram_parameter("d1W", [D, D], BF16, isOutput=False)
    bcols = nc.declare_dram_parameter("bcols", [P, 8 * KD], F32, isOutput=False)
    vber = nc.declare_dram_parameter("vber", [1, VA], BF16, isOutput=False)
    vld = nc.declare_dram_parameter("vld", [P, BL * MK], F32, isOutput=False)
    onesr = nc.declare_dram_parameter("onesr", [1, P], F32, isOutput=False)
    onesrb = nc.declare_dram_parameter("onesrb", [1, P], BF16, isOutput=False)
    onescb = nc.declare_dram_parameter("onescb", [P, 1], BF16, isOutput=False)
    onescf = nc.declare_dram_parameter("onescf", [P, 1], F32, isOutput=False)
    ident = nc.declare_dram_parameter("ident", [P, P], F32, isOutput=False)
    out = nc.declare_dram_parameter("out", [BL, NQ, D], F32, isOutput=True)

    with tile.TileContext(nc) as tc:
        with (
            tc.tile_pool(name="const", bufs=1) as cst,
            tc.tile_pool(name="gnT", bufs=2) as gnp,
            tc.tile_pool(name="big", bufs=2) as big,      # condT / per-head P~ / LN sq
            tc.tile_pool(name="kh", bufs=2) as khp,
            tc.tile_pool(name="qh", bufs=2) as qhp,
            tc.tile_pool(name="vh", bufs=2) as vhp,
            tc.tile_pool(name="xres", bufs=1) as xrp,
            tc.tile_pool(name="xbf", bufs=1) as xbp,
            tc.tile_pool(name="yy", bufs=1) as yyp,
            tc.tile_pool(name="outp", bufs=1) as otp,
            tc.tile_pool(name="ctx", bufs=1) as ctp,
            tc.tile_pool(name="zz", bufs=1) as zzp,
            tc.tile_pool(name="small", bufs=1) as sml,
            tc.tile_pool(name="sp", bufs=2, space="PSUM") as spp,   # [P,1024] 2-bank
            tc.tile_pool(name="mm", bufs=2, space="PSUM") as mmp,   # [P,512]
            tc.tile_pool(name="ms", bufs=2, space="PSUM") as msp,   # [P,512]
        ):
            # ---- resident constants -------------------------------------
            qWe_sb = cst.tile([P, KD, D], BF16, tag="qWe")
            nc.sync.dma_start(qWe_sb[:], qWe[:].rearrange("(ko p) n -> p ko n", p=P))
            kWe_sb = cst.tile([P, KL, D], BF16, tag="kWe")
            nc.sync.dma_start(kWe_sb[:], kWe[:].rearrange("(ko p) n -> p ko n", p=P))
            vWe_sb = cst.tile([P, KL, VA], BF16, tag="vWe")
            nc.sync.dma_start(vWe_sb[:], vWe[:].rearrange("(ko p) n -> p ko n", p=P))
            outW_sb = cst.tile([P, KD, D], BF16, tag="outW")
            nc.sync.dma_start(outW_sb[:], outW[:].rearrange("(ko p) n -> p ko n", p=P))
            d1W_sb = cst.tile([P, KD, D], BF16, tag="d1W")
            nc.sync.dma_start(d1W_sb[:], d1W[:].rearrange("(ko p) n -> p ko n", p=P))
            bc_sb = cst.tile([P, 8 * KD], F32, tag="bcols")
            nc.sync.dma_start(bc_sb[:], bcols[:])
            vber_sb = cst.tile([1, VA], BF16, tag="vber")
            nc.sync.dma_start(vber_sb[:], vber[:])
            vld_sb = cst.tile([P, BL * MK], F32, tag="vld")
            nc.sync.dma_start(vld_sb[:], vld[:])
            onesr_sb = cst.tile([1, P], F32, tag="onesr")
            nc.sync.dma_start(onesr_sb[:], onesr[:])
            onesrb_sb = cst.tile([1, P], BF16, tag="onesrb")
            nc.sync.dma_start(onesrb_sb[:], onesrb[:])
            onescb_sb = cst.tile([P, 1], BF16, tag="onescb")
            nc.sync.dma_start(onescb_sb[:], onescb[:])
            onescf_sb = cst.tile([P, 1], F32, tag="onescf")
            nc.sync.dma_start(onescf_sb[:], onescf[:])
            id_sb = cst.tile([P, P], F32, tag="ident")
            nc.sync.dma_start(id_sb[:], ident[:])

            qbe_c = bc_sb[:, 0 * KD:1 * KD]
            kbe_c = bc_sb[:, 1 * KD:2 * KD]
            outb_c = bc_sb[:, 2 * KD:3 * KD]
            d1b_c = bc_sb[:, 3 * KD:4 * KD]
            ln1g_c = bc_sb[:, 4 * KD:5 * KD]
            ln1b_c = bc_sb[:, 5 * KD:6 * KD]
            ln2g_c = bc_sb[:, 6 * KD:7 * KD]
            ln2b_c = bc_sb[:, 7 * KD:8 * KD]

            def layer_norm(x_sb, g_c, b_c, out_sb):
                """Feature-major LN over partitions (768 feats = 6 chunks).
                Stats fp32; dtypes follow x_sb."""
                fp = x_sb.dtype == F32
                ones = onescf_sb if fp else onescb_sb
                sum1 = msp.tile([P, NQ], F32, tag="ms")
                sq_sb = big.tile([P, KD, NQ], BF16, tag="big")
                sum2 = msp.tile([P, NQ], F32, tag="ms")
                for kc in range(KD):
                    nc.tensor.matmul(sum1[0:1, :], ones[:], x_sb[:, kc, :],
                                     start=(kc == 0), stop=(kc == KD - 1))
                    nc.scalar.activation(sq_sb[:, kc, :], x_sb[:, kc, :], AF.Square)
                    nc.tensor.matmul(sum2[0:1, :], onescb_sb[:], sq_sb[:, kc, :],
                                     start=(kc == 0), stop=(kc == KD - 1))
                m_sb = sml.tile([1, NQ], F32, tag="m")
                nc.vector.tensor_scalar_mul(m_sb[:], sum1[0:1, :], 1.0 / D)
                e2_sb = sml.tile([1, NQ], F32, tag="e2")
                nc.vector.tensor_scalar(e2_sb[:], sum2[0:1, :], 1.0 / D, EPS,
                                        ALU.mult, ALU.add)
                msq_sb = sml.tile([1, NQ], F32, tag="msq")
                nc.vector.tensor_tensor(msq_sb[:], m_sb[:], m_sb[:], ALU.mult)
                nc.vector.tensor_tensor(e2_sb[:], e2_sb[:], msq_sb[:], ALU.subtract)
                sd_sb = sml.tile([1, NQ], F32, tag="sd")
                nc.scalar.activation(sd_sb[:], e2_sb[:], AF.Sqrt)
                rs_sb = sml.tile([1, NQ], F32, tag="rs")
                nc.vector.reciprocal(rs_sb[:], sd_sb[:])
                m_bc = msp.tile([P, NQ], F32, tag="ms")
                nc.tensor.matmul(m_bc[:], onesr_sb[:], m_sb[:], start=True, stop=True)
                rs_bc = msp.tile([P, NQ], F32, tag="ms")
                nc.tensor.matmul(rs_bc[:], onesr_sb[:], rs_sb[:], start=True, stop=True)
                for kc in range(KD):
                    t_sb = sml.tile([P, NQ], F32, tag="lnt")
                    nc.vector.tensor_tensor(t_sb[:], x_sb[:, kc, :], m_bc[:], ALU.subtract)
                    nc.vector.tensor_tensor(t_sb[:], t_sb[:], rs_bc[:], ALU.mult)
                    nc.vector.tensor_scalar(out_sb[:, kc, :], t_sb[:],
                                            g_c[:, kc:kc + 1], b_c[:, kc:kc + 1],
                                            ALU.mult, ALU.add)

            for b in range(BL):
                # ---- input DMAs ----------------------------------------
                gnT_sb = gnp.tile([P, KD, NQ], BF16, tag="gnT")
                nc.sync.dma_start(gnT_sb[:], gnT[b].rearrange("(ko p) t -> p ko t", p=P))
                gnTf_sb = zzp.tile([P, KD, NQ], F32, tag="zz")
                nc.sync.dma_start(gnTf_sb[:], gnTf[b].rearrange("(ko p) t -> p ko t", p=P))
                condT_sb = big.tile([P, KL, NK], BF16, tag="big")
                nc.sync.dma_start(condT_sb[:], condT[b].rearrange("(ko p) t -> p ko t", p=P))

                # ---- projections (feature-major, bf16 outputs) ----------
                qhT_sb = qhp.tile([P, KD, NQ], BF16, tag="qh")
                for m in range(KD):
                    ps = mmp.tile([P, 512], F32, tag="mm")
                    for kc in range(KD):
                        nc.tensor.matmul(ps[:, :NQ], qWe_sb[:, kc, m * P:(m + 1) * P],
                                         gnT_sb[:, kc, :], start=(kc == 0), stop=(kc == KD - 1))
                    nc.vector.tensor_scalar_add(qhT_sb[:, m, :], ps[:, :NQ], qbe_c[:, m:m + 1])

                khT_sb = khp.tile([P, KD, NK], BF16, tag="kh")
                for m in range(KD):
                    ps = spp.tile([P, NK], F32, tag="sp")
                    for n in range(2):
                        for kc in range(KL):
                            nc.tensor.matmul(ps[:, n * 512:(n + 1) * 512],
                                             kWe_sb[:, kc, m * P:(m + 1) * P],
                                             condT_sb[:, kc, n * 512:(n + 1) * 512],
                                             start=(kc == 0), stop=(kc == KL - 1))
                    nc.vector.tensor_scalar_add(khT_sb[:, m, :], ps[:], kbe_c[:, m:m + 1])

                vh_sb = vhp.tile([P, MK, VA], BF16, tag="vh")
                for mk in range(MK):
                    ps = spp.tile([P, NK], F32, tag="sp")
                    for (n0, nw) in ((0, 512), (512, VA - 512)):
                        for kc in range(KL):
                            nc.tensor.matmul(ps[:, n0:n0 + nw],
                                             condT_sb[:, kc, mk * P:(mk + 1) * P],
                                             vWe_sb[:, kc, n0:n0 + nw],
                                             start=(kc == 0), stop=False)
                        nc.tensor.matmul(ps[:, n0:n0 + nw], onesrb_sb[:],
                                         vber_sb[:, n0:n0 + nw], start=False, stop=True)
                    # bias included; now zero masked key rows (incl ones-col)
                    nc.scalar.activation(vh_sb[:, mk, :], ps[:, :VA], AF.Copy,
                                         scale=vld_sb[:, b * MK + mk: b * MK + mk + 1])

                # ---- attention (head pairs, feature-major) --------------
                ctxT_sb = ctp.tile([P, KD, NQ], BF16, tag="ctx")
                def s_block(hp, pT):
                    th = hp
                    for g2 in range(MK // 2):
                        s_ps = [spp.tile([P, NK], F32, tag="sp", name=f"s_ps{e}") for e in range(2)]
                        for half in range(2):
                            mk = 2 * g2 + half
                            for e in range(2):   # even/odd head interleaved
                                off = DH * e
                                nc.tensor.matmul(s_ps[e][:, half * NQ:(half + 1) * NQ],
                                                 khT_sb[off:off + DH, th, mk * P:(mk + 1) * P],
                                                 qhT_sb[off:off + DH, th, :],
                                                 start=True, stop=True)
                        for e in range(2):
                            nc.scalar.activation(pT[e][:, 2 * g2:2 * g2 + 2, :], s_ps[e][:], AF.Exp)

                for hp in range(H // 2):
                    th = hp
                    pT = [big.tile([P, MK, NQ], BF16, tag="big", name=f"pT{e}") for e in range(2)]
                    s_block(hp, pT)
                    for e in range(2):
                        h = 2 * hp + e
                        off = DH * e
                        c_ps = mmp.tile([P, 512], F32, tag="mm")
                        for kc in range(MK):
                            nc.tensor.matmul(c_ps[0:DH + 1, :NQ],
                                             vh_sb[:, kc, h * (DH + 1):(h + 1) * (DH + 1)],
                                             pT[e][:, kc, :],
                                             start=(kc == 0), stop=(kc == MK - 1))
                        # softmax denominator -> reciprocal -> PE broadcast
                        dtmp = sml.tile([1, NQ], F32, tag="dtmp")
                        nc.scalar.copy(dtmp[:], c_ps[DH:DH + 1, :NQ])
                        rtmp = sml.tile([1, NQ], F32, tag="rtmp")
                        nc.vector.reciprocal(rtmp[:], dtmp[:])
                        r_ps = mmp.tile([P, 512], F32, tag="mm")
                        nc.tensor.matmul(r_ps[0:DH, :NQ], onesr_sb[0:1, 0:DH], rtmp[:],
                                         start=True, stop=True)
                        nc.vector.tensor_copy(ctxT_sb[off:off + DH, th, :], c_ps[0:DH, :NQ])
                        nc.vector.tensor_tensor(ctxT_sb[off:off + DH, th, :],
                                                ctxT_sb[off:off + DH, th, :],
                                                r_ps[0:DH, :NQ], ALU.mult)

                # ---- out-proj + residual + LN1 --------------------------
                xres_sb = xrp.tile([P, KD, NQ], F32, tag="xres")
                for m in range(KD):
                    ps = mmp.tile([P, 512], F32, tag="mm")
                    for kc in range(KD):
                        nc.tensor.matmul(ps[:, :NQ], outW_sb[:, kc, m * P:(m + 1) * P],
                                         ctxT_sb[:, kc, :], start=(kc == 0), stop=(kc == KD - 1))
                    t_sb = sml.tile([P, NQ], F32, tag="lnt")
                    nc.vector.tensor_scalar_add(t_sb[:], ps[:, :NQ], outb_c[:, m:m + 1])
                    nc.vector.tensor_tensor(xres_sb[:, m, :], t_sb[:],
                                            gnTf_sb[:, m, :], ALU.add)
                layer_norm(xres_sb, ln1g_c, ln1b_c, xres_sb)
                xbf_sb = xbp.tile([P, KD, NQ], BF16, tag="xbf")
                for m in range(KD):
                    nc.vector.tensor_copy(xbf_sb[:, m, :], xres_sb[:, m, :])

                # ---- FFN: y = leaky_relu(x @ d1W + d1b) + x, then LN2 ----
                y_sb = yyp.tile([P, KD, NQ], F32, tag="yy")
                for m in range(KD):
                    ps = mmp.tile([P, 512], F32, tag="mm")
                    for kc in range(KD):
                        nc.tensor.matmul(ps[:, :NQ], d1W_sb[:, kc, m * P:(m + 1) * P],
                                         xbf_sb[:, kc, :], start=(kc == 0), stop=(kc == KD - 1))
                    t_sb = sml.tile([P, NQ], F32, tag="lnt")
                    nc.scalar.activation(t_sb[:], ps[:, :NQ], AF.Lrelu,
                                         bias=d1b_c[:, m:m + 1], alpha=0.01)
                    nc.vector.tensor_tensor(y_sb[:, m, :], t_sb[:],
                                            xres_sb[:, m, :], ALU.add)
                z_sb = zzp.tile([P, KD, NQ], F32, tag="zz")
                layer_norm(y_sb, ln2g_c, ln2b_c, z_sb)

                # ---- transpose back to [tok, feat] and store ------------
                out_sb = otp.tile([P, MQ, D], F32, tag="outp")
                for m in range(KD):
                    for t in range(MQ):
                        tr_ps = mmp.tile([P, 512], F32, tag="mm")
                        nc.tensor.transpose(tr_ps[:, :P], z_sb[:, m, t * P:(t + 1) * P], id_sb[:])
                        nc.vector.tensor_copy(out_sb[:, t, m * P:(m + 1) * P], tr_ps[:, :P])
                nc.sync.dma_start(out[b].rearrange("(to p) f -> p to f", p=P), out_sb[:])

    nc.compile()
    return nc


def kernel(**inputs):
    gn = np.asarray(inputs["graph_nodes"], dtype=np.float32)
    cond = np.asarray(inputs["conditioning_vector"], dtype=np.float32)
    mask = np.asarray(inputs["conditioning_attention_mask"])
    g = lambda k: np.asarray(inputs[k], dtype=np.float32)

    qW, qb = g("qW"), g("qb")
    kW, kb = g("kW"), g("kb")
    vW, vb = g("vW"), g("vb")
    in_qW, in_qb = g("in_qW"), g("in_qb")
    in_kW, in_kb = g("in_kW"), g("in_kb")
    in_vW, in_vb = g("in_vW"), g("in_vb")
    outW, outb = g("outW"), g("outb")
    ln1g, ln1b = g("ln1g"), g("ln1b")
    d1W, d1b = g("d1W"), g("d1b")
    ln2g, ln2b = g("ln2g"), g("ln2b")

    scale = 1.0 / np.sqrt(np.float32(DH))
    qWe = (qW @ in_qW) * scale
    qbe = (qb @ in_qW + in_qb) * scale
    kWe = kW @ in_kW
    kbe = kb @ in_kW + in_kb
    vWe = vW @ in_vW
    vbe = vb @ in_vW + in_vb

    # vWe augmented with a zero column per head; bias row carries vbe + ones
    vWe_aug = np.zeros((L, VA), np.float32)
    vbe_aug = np.zeros((VA,), np.float32)
    for h in range(H):
        vWe_aug[:, h * (DH + 1):h * (DH + 1) + DH] = vWe[:, h * DH:(h + 1) * DH]
        vbe_aug[h * (DH + 1):h * (DH + 1) + DH] = vbe[h * DH:(h + 1) * DH]
        vbe_aug[h * (DH + 1) + DH] = 1.0

    col = lambda v: np.ascontiguousarray(v.reshape(KD, P).T, dtype=np.float32)  # [P, KD]
    bcols = np.concatenate(
        [col(qbe), col(kbe), col(outb), col(d1b),
         col(ln1g), col(ln1b), col(ln2g), col(ln2b)], axis=1)

    valid01 = np.where(mask, 0.0, 1.0).astype(np.float32)  # [B, NK]

    key = "nc"
    if key not in _NC_CACHE:
        _NC_CACHE[key] = _build_nc()
    nc = _NC_CACHE[key]

    bf = lambda a: np.ascontiguousarray(a.astype(NPBF))
    shared = {
        "qWe": bf(qWe), "kWe": bf(kWe), "vWe": bf(vWe_aug),
        "outW": bf(outW), "d1W": bf(d1W),
        "bcols": np.ascontiguousarray(bcols),
        "vber": bf(vbe_aug[None, :]),
        "onesr": np.ones((1, P), np.float32),
        "onesrb": np.ones((1, P), NPBF),
        "onescb": np.ones((P, 1), NPBF),
        "onescf": np.ones((P, 1), np.float32),
        "ident": np.eye(P, dtype=np.float32),
    }
    in_maps = []
    for c in range(NCORES):
        bs = slice(c * BL, (c + 1) * BL)
        vp = np.zeros((P, BL * MK), np.float32)
        for i, bb in enumerate(range(c * BL, (c + 1) * BL)):
            vp[:, i * MK:(i + 1) * MK] = valid01[bb].reshape(MK, P).T
        in_maps.append({
            **shared,
            "gnT": bf(gn[bs].transpose(0, 2, 1)),
            "gnTf": np.ascontiguousarray(gn[bs].transpose(0, 2, 1)),
            "condT": bf(cond[bs].transpose(0, 2, 1)),
            "vld": vp,
        })

    res = run_bass_kernel_spmd(nc, in_maps, list(range(NCORES)))
    return np.concatenate([res.results[c]["out"] for c in range(NCORES)], axis=0)
